# revision 1
# baseline (speedup 1.0000x reference)
"""Trainium2 Bass kernel for nn_MultiHeadAttention_54211077210696.

8-core SPMD sharding: batch (2-way) x heads (4-way).
Core c (b = c//4, j = c%4) computes heads 4j..4j+3 of batch b.

v3 design:
  - Q^T/K^T projections in head-pair layout [128, S] (head h of pair at
    partitions h*64..h*64+64) -- no zero padding.
  - Scores via 64-contraction matmuls, row-tiled: the two heads of a pair
    run CONCURRENTLY on the PE array (tile_position derived from base
    partitions), halving score time.
  - PSUM as 3 rotating [128,1024] score slots (one kt: 2 heads x 512q)
    + 2 x [128,512] caug accumulators; exp per slot on ScalarE; scores
    stay 2 slots ahead of PV so the ACT queue never starves.
  - Softmax denominator from an all-ones block appended to V.
  - q processed in 4 chunks of 512; out-projection + fp16 ReduceScatter
    per chunk, interleaved into the next chunk's sweep; LayerNorms for
    early chunks interleaved too (rstd = exp(-0.5*ln(var+eps)) keeps Ln
    and Exp in ONE ACT table set -- no table switches ever).
  - x tensors fully resident, whole-chunk DMAs round-robined on the 3
    HWDGE queues in need-order, no ring gating.
Matmuls fp16 with fp32 PSUM accumulation.
"""

import numpy as np
from contextlib import ExitStack

import concourse.bass as bass  # noqa: F401  (registers bass types)
import concourse.tile as tile
from concourse import bacc, mybir
from concourse.bass_utils import run_bass_kernel_spmd

F32 = mybir.dt.float32
F16 = mybir.dt.float16
AF = mybir.ActivationFunctionType
ALU = mybir.AluOpType

_NC = None

N_CORES = 8
GROUPS = [[0, 1, 2, 3], [4, 5, 6, 7]]
B, S, DM = 2, 2048, 1024
HL = 4          # heads per core
DT = 2          # head pairs per core
KT16 = 16       # 128-row k tiles
NCH = 4         # q chunks
QC = 512        # q chunk width
EPS = 1e-5


def _layernorm_c(nc, t, c, lnp, stp, resid_sb, eps_t):
    # ro16 rides the sync queue: the gpsimd queue blocks on in-flight
    # collectives, which would stall every DVE op queued behind this load.
    ro16 = lnp.tile([128, DM], F16, tag="ro16", name="ro16")
    nc.sync.dma_start(ro16[:], t["rs_out"][c].ap())
    of = lnp.tile([128, DM], F32, tag="of", name="of")
    nc.vector.tensor_copy(of[:], ro16[:])
    orow = lnp.tile([128, DM], F32, tag="orow", name="orow")
    nc.vector.tensor_add(orow[:], of[:], resid_sb[c][:])
    stats = stp.tile([128, 2, 6], F32, tag="st", name="st")
    for i in range(2):
        nc.vector.bn_stats(stats[:, i, :], orow[:, i * 512:(i + 1) * 512])
    mv = stp.tile([128, 2], F32, tag="mv", name="mv")
    nc.vector.bn_aggr(mv[:], stats[:])
    # rstd = exp(-0.5*ln(var+eps)): Ln and Exp share one ACT table set.
    lnv = stp.tile([128, 1], F32, tag="lnv", name="lnv")
    nc.scalar.activation(lnv[:], mv[:, 1:2], AF.Ln, bias=eps_t[:], scale=1.0)
    rstd = stp.tile([128, 1], F32, tag="rstd", name="rstd")
    nc.scalar.activation(rstd[:], lnv[:], AF.Exp, scale=-0.5)
    nc.vector.tensor_scalar(orow[:], orow[:], mv[:, 0:1], rstd[:],
                            ALU.subtract, ALU.mult)
    nc.sync.dma_start(t["out"][c][:, 0:512], orow[:, 0:512])
    nc.scalar.dma_start(t["out"][c][:, 512:1024], orow[:, 512:1024])


def _emit(nc, tc, ctx, t):
    sing = ctx.enter_context(tc.tile_pool(name="sing", bufs=1))
    eop = ctx.enter_context(tc.tile_pool(name="eop", bufs=4))
    ctp = ctx.enter_context(tc.tile_pool(name="ctp", bufs=4))
    rp = ctx.enter_context(tc.tile_pool(name="rp", bufs=2))
    ostp = ctx.enter_context(tc.tile_pool(name="ostp", bufs=2))
    lnp = ctx.enter_context(tc.tile_pool(name="lnp", bufs=2))
    stp = ctx.enter_context(tc.tile_pool(name="stp", bufs=4))
    psp = ctx.enter_context(tc.tile_pool(name="psp", bufs=1, space="PSUM"))

    # persistent SBUF tiles
    QTt = [sing.tile([128, S], F16, tag=f"qt{d}", name=f"qt{d}") for d in range(DT)]
    KTt = [sing.tile([128, S], F16, tag=f"kt{d}", name=f"kt{d}") for d in range(DT)]
    vaug = sing.tile([128, HL * KT16 * 128], F16, tag="vaug")
    wo_sb = [sing.tile([128, DM], F16, tag=f"wo{p}", name=f"wo{p}") for p in range(DT)]
    resid_sb = [sing.tile([128, DM], F32, tag=f"res{c}", name=f"res{c}")
                for c in range(NCH)]
    eps_t = sing.tile([128, 1], F32, tag="eps")
    wkt = [sing.tile([128, 256], F16, tag=f"wk{i}", name=f"wk{i}") for i in range(8)]
    wqt = [sing.tile([128, 256], F16, tag=f"wq{i}", name=f"wq{i}") for i in range(8)]
    wvt = [sing.tile([128, 256], F16, tag=f"wv{i}", name=f"wv{i}") for i in range(8)]
    xk_res = [sing.tile([128, S], F16, tag=f"xk{i}", name=f"xk{i}") for i in range(8)]
    xq_res = [sing.tile([128, S], F16, tag=f"xq{i}", name=f"xq{i}") for i in range(8)]
    xv_res = [sing.tile([128, S], F16, tag=f"xv{i}", name=f"xv{i}") for i in range(8)]

    nc.vector.memset(eps_t[:], EPS)
    warm = sing.tile([128, 1], F32, tag="warm")
    nc.scalar.activation(warm[:], eps_t[:], AF.Exp, scale=1.0)
    nc.gpsimd.memset(vaug[:], 1.0)

    # ---- DMA prefetch: whole-chunk transfers, need-order, no gating ----
    qrot = [nc.sync, nc.scalar, nc.gpsimd]
    for i in range(8):
        nc.scalar.dma_start(wkt[i][:], t["wk"][i * 128:(i + 1) * 128, :])
    for i in range(8):
        qrot[i % 3].dma_start(xk_res[i][:], t["xkT"][i * 128:(i + 1) * 128, :])
    for i in range(8):
        nc.gpsimd.dma_start(wqt[i][:], t["wq"][i * 128:(i + 1) * 128, :])
    for i in range(8):
        qrot[i % 3].dma_start(xq_res[i][:], t["xqT"][i * 128:(i + 1) * 128, :])
    for i in range(8):
        nc.gpsimd.dma_start(wvt[i][:], t["wv"][i * 128:(i + 1) * 128, :])
    for p in range(DT):
        nc.gpsimd.dma_start(wo_sb[p][:], t["wo"][p * 128:(p + 1) * 128, :])
    for i in range(8):
        qrot[i % 3].dma_start(xv_res[i][:], t["xvT"][i * 128:(i + 1) * 128, :])
    for c in range(NCH):
        nc.gpsimd.dma_start(resid_sb[c][:], t["resid"][c])

    # Tiny dummy collective: absorbs the ~40us CC entry barrier + stream
    # spin-up during the QKV front so the first real RS completes fast.
    nc.gpsimd.collective_compute(
        "ReduceScatter", ALU.add, replica_groups=GROUPS,
        ins=[t["brx"].ap().opt()], outs=[t["bro"].ap().opt()])

    # PSUM: 3 rotating Bp slots [128,1024] (6 banks) + 2 cg [128,512].
    def psB():
        return psp.tile([128, 1024], F32, tag="Bp", name="psB", bufs=3)

    def psC(nm="psc"):
        return psp.tile([128, 512], F32, tag="cg", name=nm, bufs=2)

    # ---- PE warmup: ~5us of junk matmuls while the first DMAs land, so
    # the HAM clock-gate opens before the K projection starts.
    junk_sb = sing.tile([128, 512], F32, tag="junk")
    nc.vector.memset(junk_sb[:], 0.0)
    junkps = psC("junkps")
    for _ in range(12):
        nc.tensor.matmul(junkps[0:1, :], eps_t[:, 0:1], junk_sb[:])

    # ---- K / Q projections: 8 chains over all 8 banks ----
    def qk_sweep(xres, wt, dst):
        pb = [psB() for _ in range(3)]
        pc = [psC() for _ in range(2)]

        def chain(j):
            if j < 6:
                return pb[j // 2][:, (j % 2) * 512:(j % 2) * 512 + 512]
            return pc[j - 6][:, :]

        for dmc in range(8):
            xc = xres[dmc]
            st = dict(start=(dmc == 0), stop=(dmc == 7))
            for d in range(DT):
                for sc in range(4):
                    nc.tensor.matmul(chain(d * 4 + sc), wt[dmc][:, d * 128:(d + 1) * 128],
                                     xc[:, sc * 512:(sc + 1) * 512], **st)
        for d in range(DT):
            for sc in range(4):
                nc.vector.tensor_copy(dst[d][:, sc * 512:(sc + 1) * 512],
                                      chain(d * 4 + sc))

    qk_sweep(xk_res, wkt, KTt)
    qk_sweep(xq_res, wqt, QTt)

    # ---- V projection: 2 waves x 8 s-tile chains ----
    for wave in range(2):
        pb = [psB() for _ in range(3)]
        pc = [psC("pscv") for _ in range(2)]

        def vchain(i):
            if i < 6:
                return pb[i // 2][:, (i % 2) * 512:(i % 2) * 512 + 256]
            return pc[i - 6][:, 0:256]

        for dmc in range(8):
            for i in range(8):
                st = wave * 8 + i
                nc.tensor.matmul(vchain(i), xv_res[dmc][:, st * 128:(st + 1) * 128],
                                 wvt[dmc][:], start=(dmc == 0), stop=(dmc == 7))
        for i in range(8):
            st = wave * 8 + i
            src = vchain(i)
            for h in range(HL):
                nc.vector.tensor_copy(
                    vaug[:, (h * KT16 + st) * 128:(h * KT16 + st) * 128 + 64],
                    src[:, h * 64:h * 64 + 64])

    # ---- attention: 4 chunks x 2 pairs; per kt one [128,1024] slot ----
    def emit_pv(p, caug, eo, kt):
        for h in range(2):
            blk = ((p * 2 + h) * KT16 + kt) * 128
            nc.tensor.matmul(caug[h][:, :], vaug[:, blk:blk + 128],
                             eo[:, h * 512:(h + 1) * 512],
                             start=(kt == 0), stop=(kt == KT16 - 1))

    def sweep(c, p, hooks, finish_prev):
        """Emit one (chunk, pair) kt sweep. The PREVIOUS sweep's PV flush +
        normalize are emitted right after this sweep's first exp, so the PE
        tail of sweep i never drains the ACT queue at the boundary.
        Returns (ct, finish): ct is written when finish() has been called.
        """
        qb = c * QC
        caug = [psC(f"caug{h}") for h in range(2)]
        ct = ctp.tile([128, QC], F16, tag="ct", name="ct")
        pend = []
        hooks = dict(hooks)
        for kt in range(KT16):
            ps = psB()
            for h in range(2):
                nc.tensor.matmul(
                    ps[:, h * 512:(h + 1) * 512],
                    KTt[p][h * 64:(h + 1) * 64, kt * 128:(kt + 1) * 128],
                    QTt[p][h * 64:(h + 1) * 64, qb:qb + QC])
            eo = eop.tile([128, 1024], F16, tag="eo", name="eo")
            nc.scalar.activation(eo[:], ps[:], AF.Exp, scale=0.125)
            pend.append((eo, kt))
            if kt == 1 and finish_prev is not None:
                # two slots of lookahead are already queued on ACT, so the
                # previous sweep's PE flush hides under them
                finish_prev()
            if len(pend) > 2:
                emit_pv(p, caug, *pend.pop(0))
            if kt in hooks:
                for fn in hooks[kt]:
                    fn()

        def finish():
            for pe in pend:
                emit_pv(p, caug, *pe)
            # normalize: rows 64..127 of caug hold the softmax denominator
            for h in range(2):
                rt = rp.tile([64, QC], F32, tag="rt", name="rt")
                nc.vector.tensor_copy(rt[:], caug[h][64:128, :])
                rt2 = rp.tile([64, QC], F32, tag="rt2", name="rt2")
                nc.vector.reciprocal_approx_fast(rt2[:], rt[:])
                nc.vector.tensor_mul(ct[h * 64:(h + 1) * 64, :],
                                     caug[h][0:64, :], rt2[:])
        return ct, finish

    def outproj_piece(c, cts, qt):
        po = psB()
        for p in range(DT):
            for dmc in range(2):
                nc.tensor.matmul(po[:, dmc * 512:(dmc + 1) * 512],
                                 cts[p][:, qt * 128:(qt + 1) * 128],
                                 wo_sb[p][:, dmc * 512:(dmc + 1) * 512],
                                 start=(p == 0), stop=(p == DT - 1))
        ost = ostp.tile([128, 1024], F16, tag="ost", name="ost")
        nc.vector.tensor_copy(ost[:], po[:])
        nc.sync.dma_start(t["rs_in"][c][qt * 128:(qt + 1) * 128, :], ost[:])

    def fire_rs(c):
        nc.gpsimd.collective_compute(
            "ReduceScatter", ALU.add, replica_groups=GROUPS,
            ins=[t["rs_in"][c].ap().opt()], outs=[t["rs_out"][c].ap().opt()])

    cts_prev = None
    c_prev = None
    fin_prev = None
    for c in range(NCH):
        cts_cur = {}
        for p in range(DT):
            hooks = []
            if p == 0 and cts_prev is not None:
                cp, cc = c_prev, dict(cts_prev)
                hooks = [(2, [lambda cp=cp, cc=cc: outproj_piece(cp, cc, 0)]),
                         (5, [lambda cp=cp, cc=cc: outproj_piece(cp, cc, 1)]),
                         (8, [lambda cp=cp, cc=cc: outproj_piece(cp, cc, 2)]),
                         (11, [lambda cp=cp, cc=cc: outproj_piece(cp, cc, 3)]),
                         (13, [lambda cp=cp: fire_rs(cp)])]
            cts_cur[p], fin_prev = sweep(c, p, hooks, fin_prev)
        cts_prev, c_prev = cts_cur, c
    fin_prev()

    # ---- tail: last chunk out-proj, fire RS_3, then all LayerNorms,
    # batched: prefetch every rs_out shard first, residual-add in place
    # into resid_sb, then ALL Ln's followed by ALL Exp's so the ACT table
    # switches twice total instead of per-LN. wait_until pins this past
    # the loop so the scheduler cannot hoist it.
    for qt in range(4):
        outproj_piece(c_prev, cts_prev, qt)
    fire_rs(c_prev)
    with tc.tile_wait_until(2.0):
        ro16s, mvs = [], []
        for c in range(NCH):
            ro = lnp.tile([128, DM], F16, tag="ro16", name="ro16", bufs=4)
            nc.sync.dma_start(ro[:], t["rs_out"][c].ap())
            ro16s.append(ro)
        for c in range(NCH):
            nc.vector.tensor_add(resid_sb[c][:], ro16s[c][:], resid_sb[c][:])
            stats = stp.tile([128, 2, 6], F32, tag="st", name="st")
            for i in range(2):
                nc.vector.bn_stats(stats[:, i, :],
                                   resid_sb[c][:, i * 512:(i + 1) * 512])
            mv = stp.tile([128, 2], F32, tag="mv", name="mv")
            nc.vector.bn_aggr(mv[:], stats[:])
            mvs.append(mv)
        # chunks 0-2: three Ln's into one tile + ONE 3-wide Exp (2 table
        # switches, not 6), finished while RS_3 is still in flight; then
        # chunk 3 alone once its shard lands.
        lnv3 = stp.tile([128, 4], F32, tag="lnv", name="lnv3")
        for c in range(3):
            nc.scalar.activation(lnv3[:, c:c + 1], mvs[c][:, 1:2], AF.Ln,
                                 bias=eps_t[:], scale=1.0)
        rstd3 = stp.tile([128, 4], F32, tag="rstd", name="rstd3")
        nc.scalar.activation(rstd3[:, 0:3], lnv3[:, 0:3], AF.Exp, scale=-0.5)
        for c in range(3):
            nc.vector.tensor_scalar(resid_sb[c][:], resid_sb[c][:],
                                    mvs[c][:, 0:1], rstd3[:, c:c + 1],
                                    ALU.subtract, ALU.mult)
            nc.sync.dma_start(t["out"][c][:, 0:512], resid_sb[c][:, 0:512])
            nc.scalar.dma_start(t["out"][c][:, 512:1024],
                                resid_sb[c][:, 512:1024])
        lnv = stp.tile([128, 1], F32, tag="lnv", name="lnv")
        nc.scalar.activation(lnv[:], mvs[3][:, 1:2], AF.Ln,
                             bias=eps_t[:], scale=1.0)
        rstd = stp.tile([128, 1], F32, tag="rstd", name="rstd")
        nc.scalar.activation(rstd[:], lnv[:], AF.Exp, scale=-0.5)
        nc.vector.tensor_scalar(resid_sb[3][:], resid_sb[3][:],
                                mvs[3][:, 0:1], rstd[:],
                                ALU.subtract, ALU.mult)
        nc.sync.dma_start(t["out"][3][:, 0:512], resid_sb[3][:, 0:512])
        nc.scalar.dma_start(t["out"][3][:, 512:1024], resid_sb[3][:, 512:1024])


def _build():
    nc = bacc.Bacc("TRN2", target_bir_lowering=False, debug=False,
                   num_devices=N_CORES)
    t = {}
    for name in ("xqT", "xkT", "xvT"):
        t[name] = nc.dram_tensor(name, [DM, S], F16, kind="ExternalInput").ap()
    for name in ("wq", "wk", "wv"):
        t[name] = nc.dram_tensor(name, [DM, 256], F16, kind="ExternalInput").ap()
    t["wo"] = nc.dram_tensor("wo", [256, DM], F16, kind="ExternalInput").ap()
    t["resid"] = nc.dram_tensor("resid", [NCH, 128, DM], F32, kind="ExternalInput").ap()
    t["out"] = nc.dram_tensor("out", [NCH, 128, DM], F32, kind="ExternalOutput").ap()
    t["rs_in"] = [nc.dram_tensor(f"rs_in{c}", [512, DM], F16) for c in range(NCH)]
    t["rs_out"] = [nc.dram_tensor(f"rs_out{c}", [128, DM], F16) for c in range(NCH)]
    t["brx"] = nc.dram_tensor("brx", [8, 128], F16)
    t["bro"] = nc.dram_tensor("bro", [2, 128], F16)

    with tile.TileContext(nc) as tc:
        with ExitStack() as ctx:
            _emit(nc, tc, ctx, t)
    nc.compile()
    return nc


def kernel(input_Q, input_K, input_V, W_Q, W_K, W_V, W_O):
    global _NC
    if _NC is None:
        _NC = _build()
    nc = _NC

    input_Q = np.asarray(input_Q, dtype=np.float32)
    input_K = np.asarray(input_K, dtype=np.float32)
    input_V = np.asarray(input_V, dtype=np.float32)
    W_Q = np.asarray(W_Q, dtype=np.float32)
    W_K = np.asarray(W_K, dtype=np.float32)
    W_V = np.asarray(W_V, dtype=np.float32)
    W_O = np.asarray(W_O, dtype=np.float32)

    xT = {}
    for nm, x in (("q", input_Q), ("k", input_K), ("v", input_V)):
        for b in range(B):
            xT[nm, b] = np.ascontiguousarray(x[b].T).astype(np.float16)
    in_maps = []
    for core in range(N_CORES):
        b, j = core // 4, core % 4
        resid = np.empty((NCH, 128, DM), dtype=np.float32)
        for c in range(NCH):
            r0 = c * 512 + j * 128
            resid[c] = input_Q[b, r0:r0 + 128, :]
        in_maps.append({
            "xqT": xT["q", b], "xkT": xT["k", b], "xvT": xT["v", b],
            "wq": np.ascontiguousarray(W_Q[:, 256 * j:256 * j + 256]).astype(np.float16),
            "wk": np.ascontiguousarray(W_K[:, 256 * j:256 * j + 256]).astype(np.float16),
            "wv": np.ascontiguousarray(W_V[:, 256 * j:256 * j + 256]).astype(np.float16),
            "wo": np.ascontiguousarray(W_O[256 * j:256 * j + 256, :]).astype(np.float16),
            "resid": resid,
        })

    global _last_in_maps
    _last_in_maps = in_maps
    res = run_bass_kernel_spmd(nc, in_maps, core_ids=list(range(N_CORES)))

    out = np.empty((B, S, DM), dtype=np.float32)
    for core in range(N_CORES):
        b, j = core // 4, core % 4
        o = res.results[core]["out"]
        for c in range(NCH):
            r0 = c * 512 + j * 128
            out[b, r0:r0 + 128, :] = o[c]
    return out



# revision 10
# speedup vs baseline: 1.0711x; 1.0711x over previous
"""Trainium2 Bass kernel for nn_MultiHeadAttention_54211077210696.

8-core SPMD sharding: batch (2-way) x heads (4-way).
Core c (b = c//4, j = c%4) computes heads 4j..4j+3 of batch b.

v3 design:
  - Q^T/K^T projections in head-pair layout [128, S] (head h of pair at
    partitions h*64..h*64+64) -- no zero padding.
  - Scores via 64-contraction matmuls, row-tiled: the two heads of a pair
    run CONCURRENTLY on the PE array (tile_position derived from base
    partitions), halving score time.
  - PSUM as 3 rotating [128,1024] score slots (one kt: 2 heads x 512q)
    + 2 x [128,512] caug accumulators; exp per slot on ScalarE; scores
    stay 2 slots ahead of PV so the ACT queue never starves.
  - Softmax denominator from an all-ones block appended to V.
  - q processed in 4 chunks of 512; out-projection + fp16 ReduceScatter
    per chunk, interleaved into the next chunk's sweep; LayerNorms for
    early chunks interleaved too (rstd = exp(-0.5*ln(var+eps)) keeps Ln
    and Exp in ONE ACT table set -- no table switches ever).
  - x tensors fully resident, whole-chunk DMAs round-robined on the 3
    HWDGE queues in need-order, no ring gating.
Matmuls fp16 with fp32 PSUM accumulation.
"""

import numpy as np
from contextlib import ExitStack

import concourse.bass as bass  # noqa: F401  (registers bass types)
import concourse.tile as tile
from concourse import bacc, mybir
from concourse.bass_utils import run_bass_kernel_spmd

F32 = mybir.dt.float32
F16 = mybir.dt.float16
F8 = mybir.dt.float8e4
AF = mybir.ActivationFunctionType
ALU = mybir.AluOpType
PM = mybir.MatmulPerfMode
ESHIFT = 3.5  # exp(s/8 - ESHIFT): softmax-invariant, keeps fp8 eo finite

_NC = None

N_CORES = 8
GROUPS = [[0, 1, 2, 3], [4, 5, 6, 7]]
B, S, DM = 2, 2048, 1024
HL = 4          # heads per core
DT = 2          # head pairs per core
KT16 = 16       # 128-row k tiles
NCH = 4         # q chunks
QC = 512        # q chunk width
EPS = 1e-5


def _layernorm_c(nc, t, c, lnp, stp, resid_sb, eps_t):
    # ro16 rides the sync queue: the gpsimd queue blocks on in-flight
    # collectives, which would stall every DVE op queued behind this load.
    ro16 = lnp.tile([128, DM], F16, tag="ro16", name="ro16")
    nc.sync.dma_start(ro16[:], t["rs_out"][c].ap())
    of = lnp.tile([128, DM], F32, tag="of", name="of")
    nc.vector.tensor_copy(of[:], ro16[:])
    orow = lnp.tile([128, DM], F32, tag="orow", name="orow")
    nc.vector.tensor_add(orow[:], of[:], resid_sb[c][:])
    stats = stp.tile([128, 2, 6], F32, tag="st", name="st")
    for i in range(2):
        nc.vector.bn_stats(stats[:, i, :], orow[:, i * 512:(i + 1) * 512])
    mv = stp.tile([128, 2], F32, tag="mv", name="mv")
    nc.vector.bn_aggr(mv[:], stats[:])
    # rstd = exp(-0.5*ln(var+eps)): Ln and Exp share one ACT table set.
    lnv = stp.tile([128, 1], F32, tag="lnv", name="lnv")
    nc.scalar.activation(lnv[:], mv[:, 1:2], AF.Ln, bias=eps_t[:], scale=1.0)
    rstd = stp.tile([128, 1], F32, tag="rstd", name="rstd")
    nc.scalar.activation(rstd[:], lnv[:], AF.Exp, scale=-0.5)
    nc.vector.tensor_scalar(orow[:], orow[:], mv[:, 0:1], rstd[:],
                            ALU.subtract, ALU.mult)
    nc.sync.dma_start(t["out"][c][:, 0:512], orow[:, 0:512])
    nc.scalar.dma_start(t["out"][c][:, 512:1024], orow[:, 512:1024])


def _emit(nc, tc, ctx, t):
    sing = ctx.enter_context(tc.tile_pool(name="sing", bufs=1))
    eop = ctx.enter_context(tc.tile_pool(name="eop", bufs=4))
    ctp = ctx.enter_context(tc.tile_pool(name="ctp", bufs=4))
    rp = ctx.enter_context(tc.tile_pool(name="rp", bufs=2))
    ostp = ctx.enter_context(tc.tile_pool(name="ostp", bufs=2))
    lnp = ctx.enter_context(tc.tile_pool(name="lnp", bufs=2))
    stp = ctx.enter_context(tc.tile_pool(name="stp", bufs=4))
    psp = ctx.enter_context(tc.tile_pool(name="psp", bufs=1, space="PSUM"))

    # persistent SBUF tiles
    QTt = [sing.tile([128, S], F16, tag=f"qt{d}", name=f"qt{d}") for d in range(DT)]
    KTt = [sing.tile([128, S], F16, tag=f"kt{d}", name=f"kt{d}") for d in range(DT)]
    # fp8 V (+ones) for DoubleRow PV: [128 k, h*8+kp, kt-parity, 128 cols]
    # (same byte layout as the old flat (h*KT16+kt)*128 indexing)
    vaug = sing.tile([128, HL * 8, 2, 128], F8, tag="vaug")
    wo_sb = [sing.tile([128, DM], F16, tag=f"wo{p}", name=f"wo{p}") for p in range(DT)]
    resid_sb = [sing.tile([128, DM], F32, tag=f"res{c}", name=f"res{c}")
                for c in range(NCH)]
    eps_t = sing.tile([128, 1], F32, tag="eps")
    nbias_t = sing.tile([128, 1], F32, tag="nbias")
    wkt = [sing.tile([128, 256], F16, tag=f"wk{i}", name=f"wk{i}") for i in range(8)]
    wqt = [sing.tile([128, 256], F16, tag=f"wq{i}", name=f"wq{i}") for i in range(8)]
    wvt = [sing.tile([128, 256], F16, tag=f"wv{i}", name=f"wv{i}") for i in range(8)]
    xk_res = [sing.tile([128, S], F16, tag=f"xk{i}", name=f"xk{i}") for i in range(8)]
    xq_res = [sing.tile([128, S], F16, tag=f"xq{i}", name=f"xq{i}") for i in range(8)]
    xv_res = [sing.tile([128, S], F16, tag=f"xv{i}", name=f"xv{i}") for i in range(8)]

    nc.vector.memset(eps_t[:], EPS)
    nc.vector.memset(nbias_t[:], -ESHIFT)
    warm = sing.tile([128, 1], F32, tag="warm")
    nc.scalar.activation(warm[:], eps_t[:], AF.Exp, scale=1.0)
    nc.gpsimd.memset(vaug[:], 1.0)

    # ---- DMA prefetch: whole-chunk transfers, need-order, no gating ----
    qrot = [nc.sync, nc.scalar, nc.gpsimd]
    for i in range(8):
        nc.scalar.dma_start(wkt[i][:], t["wk"][i * 128:(i + 1) * 128, :])
    for i in range(8):
        qrot[i % 3].dma_start(xk_res[i][:], t["xkT"][i * 128:(i + 1) * 128, :])
    for i in range(8):
        nc.gpsimd.dma_start(wqt[i][:], t["wq"][i * 128:(i + 1) * 128, :])
    for i in range(8):
        qrot[i % 3].dma_start(xq_res[i][:], t["xqT"][i * 128:(i + 1) * 128, :])
    for i in range(8):
        nc.gpsimd.dma_start(wvt[i][:], t["wv"][i * 128:(i + 1) * 128, :])
    for p in range(DT):
        nc.gpsimd.dma_start(wo_sb[p][:], t["wo"][p * 128:(p + 1) * 128, :])
    for i in range(8):
        qrot[i % 3].dma_start(xv_res[i][:], t["xvT"][i * 128:(i + 1) * 128, :])
    for c in range(NCH):
        nc.gpsimd.dma_start(resid_sb[c][:], t["resid"][c])

    # Tiny dummy collective: absorbs the ~40us CC entry barrier + stream
    # spin-up during the QKV front so the first real RS completes fast.
    nc.gpsimd.collective_compute(
        "ReduceScatter", ALU.add, replica_groups=GROUPS,
        ins=[t["brx"].ap().opt()], outs=[t["bro"].ap().opt()])

    # PSUM: 3 rotating Bp slots [128,1024] (6 banks) + 2 cg [128,512].
    def psB():
        return psp.tile([128, 1024], F32, tag="Bp", name="psB", bufs=3)

    def psC(nm="psc"):
        return psp.tile([128, 512], F32, tag="cg", name=nm, bufs=2)

    # ---- PE warmup: ~5us of junk matmuls while the first DMAs land, so
    # the HAM clock-gate opens before the K projection starts.
    junk_sb = sing.tile([128, 512], F32, tag="junk")
    nc.vector.memset(junk_sb[:], 0.0)
    junkps = psC("junkps")
    for _ in range(12):
        nc.tensor.matmul(junkps[0:1, :], eps_t[:, 0:1], junk_sb[:])

    # ---- K / Q projections: 8 chains over all 8 banks ----
    def qk_sweep(xres, wt, dst):
        pb = [psB() for _ in range(3)]
        pc = [psC() for _ in range(2)]

        def chain(j):
            if j < 6:
                return pb[j // 2][:, (j % 2) * 512:(j % 2) * 512 + 512]
            return pc[j - 6][:, :]

        for dmc in range(8):
            xc = xres[dmc]
            st = dict(start=(dmc == 0), stop=(dmc == 7))
            for d in range(DT):
                for sc in range(4):
                    nc.tensor.matmul(chain(d * 4 + sc), wt[dmc][:, d * 128:(d + 1) * 128],
                                     xc[:, sc * 512:(sc + 1) * 512], **st)
        for d in range(DT):
            for sc in range(4):
                nc.vector.tensor_copy(dst[d][:, sc * 512:(sc + 1) * 512],
                                      chain(d * 4 + sc))

    qk_sweep(xk_res, wkt, KTt)
    qk_sweep(xq_res, wqt, QTt)

    # ---- V projection: 2 waves x 8 s-tile chains ----
    for wave in range(2):
        pb = [psB() for _ in range(3)]
        pc = [psC("pscv") for _ in range(2)]

        def vchain(i):
            if i < 6:
                return pb[i // 2][:, (i % 2) * 512:(i % 2) * 512 + 256]
            return pc[i - 6][:, 0:256]

        for dmc in range(8):
            for i in range(8):
                st = wave * 8 + i
                nc.tensor.matmul(vchain(i), xv_res[dmc][:, st * 128:(st + 1) * 128],
                                 wvt[dmc][:], start=(dmc == 0), stop=(dmc == 7))
        for i in range(8):
            st = wave * 8 + i
            src = vchain(i)
            for h in range(HL):
                nc.vector.tensor_copy(
                    vaug[:, h * 8 + st // 2, st % 2, 0:64],
                    src[:, h * 64:h * 64 + 64])

    # ---- attention: 4 chunks x 2 pairs; per kt one [128,1024] slot.
    # eo is fp8 in kt-pair parity layout [128, 2, 1024]; PV is a DoubleRow
    # matmul per (kt-pair, head): contraction 256 at 2x fp8 rate.
    def emit_pv(p, caug, eo, kp):
        for h in range(2):
            nc.tensor.matmul(caug[h][:, :],
                             vaug[:, (p * 2 + h) * 8 + kp, :, :],
                             eo[:, :, h * 512:(h + 1) * 512],
                             start=(kp == 0), stop=(kp == KT16 // 2 - 1),
                             perf_mode=PM.DoubleRow)

    def sweep(c, p, hooks, finish_prev):
        """Emit one (chunk, pair) kt sweep. The PREVIOUS sweep's PV flush +
        normalize are emitted right after this sweep's first exp, so the PE
        tail of sweep i never drains the ACT queue at the boundary.
        Returns (ct, finish): ct is written when finish() has been called.
        """
        qb = c * QC
        caug = [psC(f"caug{h}") for h in range(2)]
        ct = ctp.tile([128, QC], F16, tag="ct", name="ct")
        pend = []
        eo = None
        hooks = dict(hooks)
        for kt in range(KT16):
            ps = psB()
            for h in range(2):
                nc.tensor.matmul(
                    ps[:, h * 512:(h + 1) * 512],
                    KTt[p][h * 64:(h + 1) * 64, kt * 128:(kt + 1) * 128],
                    QTt[p][h * 64:(h + 1) * 64, qb:qb + QC])
            if kt % 2 == 0:
                eo = eop.tile([128, 2, 1024], F8, tag="eo", name="eo")
            nc.scalar.activation(eo[:, kt % 2, :], ps[:], AF.Exp,
                                 bias=nbias_t[:], scale=0.125)
            if kt % 2 == 1:
                pend.append((eo, kt // 2))
            if kt == 1 and finish_prev is not None:
                # two slots of lookahead are already queued on ACT, so the
                # previous sweep's PE flush hides under them
                finish_prev()
            if len(pend) > 1:
                emit_pv(p, caug, *pend.pop(0))
            if kt in hooks:
                for fn in hooks[kt]:
                    fn()

        def finish():
            for pe in pend:
                emit_pv(p, caug, *pe)
            # normalize: rows 64..127 of caug hold the softmax denominator
            for h in range(2):
                rt = rp.tile([64, QC], F32, tag="rt", name="rt")
                nc.vector.tensor_copy(rt[:], caug[h][64:128, :])
                rt2 = rp.tile([64, QC], F32, tag="rt2", name="rt2")
                nc.vector.reciprocal_approx_fast(rt2[:], rt[:])
                nc.vector.tensor_mul(ct[h * 64:(h + 1) * 64, :],
                                     caug[h][0:64, :], rt2[:])
        return ct, finish

    def outproj_piece(c, cts, qt):
        po = psB()
        for p in range(DT):
            for dmc in range(2):
                nc.tensor.matmul(po[:, dmc * 512:(dmc + 1) * 512],
                                 cts[p][:, qt * 128:(qt + 1) * 128],
                                 wo_sb[p][:, dmc * 512:(dmc + 1) * 512],
                                 start=(p == 0), stop=(p == DT - 1))
        ost = ostp.tile([128, 1024], F16, tag="ost", name="ost")
        nc.vector.tensor_copy(ost[:], po[:])
        nc.sync.dma_start(t["rs_in"][c][qt * 128:(qt + 1) * 128, :], ost[:])

    def fire_rs(c):
        nc.gpsimd.collective_compute(
            "ReduceScatter", ALU.add, replica_groups=GROUPS,
            ins=[t["rs_in"][c].ap().opt()], outs=[t["rs_out"][c].ap().opt()])

    cts_prev = None
    c_prev = None
    fin_prev = None
    for c in range(NCH):
        cts_cur = {}
        for p in range(DT):
            hooks = []
            if p == 0 and cts_prev is not None:
                cp, cc = c_prev, dict(cts_prev)
                hooks = [(2, [lambda cp=cp, cc=cc: outproj_piece(cp, cc, 0)]),
                         (5, [lambda cp=cp, cc=cc: outproj_piece(cp, cc, 1)]),
                         (8, [lambda cp=cp, cc=cc: outproj_piece(cp, cc, 2)]),
                         (11, [lambda cp=cp, cc=cc: outproj_piece(cp, cc, 3)]),
                         (13, [lambda cp=cp: fire_rs(cp)])]
            cts_cur[p], fin_prev = sweep(c, p, hooks, fin_prev)
        cts_prev, c_prev = cts_cur, c
    fin_prev()

    # ---- tail: last chunk out-proj, fire RS_3, then all LayerNorms,
    # batched: prefetch every rs_out shard first, residual-add in place
    # into resid_sb, then ALL Ln's followed by ALL Exp's so the ACT table
    # switches twice total instead of per-LN. wait_until pins this past
    # the loop so the scheduler cannot hoist it.
    for qt in range(4):
        outproj_piece(c_prev, cts_prev, qt)
    fire_rs(c_prev)
    with tc.tile_wait_until(2.0):
        ro16s, mvs = [], []
        for c in range(NCH):
            ro = lnp.tile([128, DM], F16, tag="ro16", name="ro16", bufs=4)
            nc.sync.dma_start(ro[:], t["rs_out"][c].ap())
            ro16s.append(ro)
        for c in range(NCH):
            nc.vector.tensor_add(resid_sb[c][:], ro16s[c][:], resid_sb[c][:])
            stats = stp.tile([128, 2, 6], F32, tag="st", name="st")
            for i in range(2):
                nc.vector.bn_stats(stats[:, i, :],
                                   resid_sb[c][:, i * 512:(i + 1) * 512])
            mv = stp.tile([128, 2], F32, tag="mv", name="mv")
            nc.vector.bn_aggr(mv[:], stats[:])
            mvs.append(mv)
        # chunks 0-2: three Ln's into one tile + ONE 3-wide Exp (2 table
        # switches, not 6), finished while RS_3 is still in flight; then
        # chunk 3 alone once its shard lands.
        lnv3 = stp.tile([128, 4], F32, tag="lnv", name="lnv3")
        for c in range(3):
            nc.scalar.activation(lnv3[:, c:c + 1], mvs[c][:, 1:2], AF.Ln,
                                 bias=eps_t[:], scale=1.0)
        rstd3 = stp.tile([128, 4], F32, tag="rstd", name="rstd3")
        nc.scalar.activation(rstd3[:, 0:3], lnv3[:, 0:3], AF.Exp, scale=-0.5)
        for c in range(3):
            nc.vector.tensor_scalar(resid_sb[c][:], resid_sb[c][:],
                                    mvs[c][:, 0:1], rstd3[:, c:c + 1],
                                    ALU.subtract, ALU.mult)
            nc.sync.dma_start(t["out"][c][:, 0:512], resid_sb[c][:, 0:512])
            nc.scalar.dma_start(t["out"][c][:, 512:1024],
                                resid_sb[c][:, 512:1024])
        lnv = stp.tile([128, 1], F32, tag="lnv", name="lnv")
        nc.scalar.activation(lnv[:], mvs[3][:, 1:2], AF.Ln,
                             bias=eps_t[:], scale=1.0)
        rstd = stp.tile([128, 1], F32, tag="rstd", name="rstd")
        nc.scalar.activation(rstd[:], lnv[:], AF.Exp, scale=-0.5)
        nc.vector.tensor_scalar(resid_sb[3][:], resid_sb[3][:],
                                mvs[3][:, 0:1], rstd[:],
                                ALU.subtract, ALU.mult)
        nc.sync.dma_start(t["out"][3][:, 0:512], resid_sb[3][:, 0:512])
        nc.scalar.dma_start(t["out"][3][:, 512:1024], resid_sb[3][:, 512:1024])


def _build():
    nc = bacc.Bacc("TRN2", target_bir_lowering=False, debug=False,
                   num_devices=N_CORES)
    t = {}
    for name in ("xqT", "xkT", "xvT"):
        t[name] = nc.dram_tensor(name, [DM, S], F16, kind="ExternalInput").ap()
    for name in ("wq", "wk", "wv"):
        t[name] = nc.dram_tensor(name, [DM, 256], F16, kind="ExternalInput").ap()
    t["wo"] = nc.dram_tensor("wo", [256, DM], F16, kind="ExternalInput").ap()
    t["resid"] = nc.dram_tensor("resid", [NCH, 128, DM], F32, kind="ExternalInput").ap()
    t["out"] = nc.dram_tensor("out", [NCH, 128, DM], F32, kind="ExternalOutput").ap()
    t["rs_in"] = [nc.dram_tensor(f"rs_in{c}", [512, DM], F16) for c in range(NCH)]
    t["rs_out"] = [nc.dram_tensor(f"rs_out{c}", [128, DM], F16) for c in range(NCH)]
    t["brx"] = nc.dram_tensor("brx", [8, 128], F16)
    t["bro"] = nc.dram_tensor("bro", [2, 128], F16)

    with tile.TileContext(nc) as tc:
        with ExitStack() as ctx:
            _emit(nc, tc, ctx, t)
    nc.compile()
    return nc


def kernel(input_Q, input_K, input_V, W_Q, W_K, W_V, W_O):
    global _NC
    if _NC is None:
        _NC = _build()
    nc = _NC

    input_Q = np.asarray(input_Q, dtype=np.float32)
    input_K = np.asarray(input_K, dtype=np.float32)
    input_V = np.asarray(input_V, dtype=np.float32)
    W_Q = np.asarray(W_Q, dtype=np.float32)
    W_K = np.asarray(W_K, dtype=np.float32)
    W_V = np.asarray(W_V, dtype=np.float32)
    W_O = np.asarray(W_O, dtype=np.float32)

    xT = {}
    for nm, x in (("q", input_Q), ("k", input_K), ("v", input_V)):
        for b in range(B):
            xT[nm, b] = np.ascontiguousarray(x[b].T).astype(np.float16)
    in_maps = []
    for core in range(N_CORES):
        b, j = core // 4, core % 4
        resid = np.empty((NCH, 128, DM), dtype=np.float32)
        for c in range(NCH):
            r0 = c * 512 + j * 128
            resid[c] = input_Q[b, r0:r0 + 128, :]
        in_maps.append({
            "xqT": xT["q", b], "xkT": xT["k", b], "xvT": xT["v", b],
            "wq": np.ascontiguousarray(W_Q[:, 256 * j:256 * j + 256]).astype(np.float16),
            "wk": np.ascontiguousarray(W_K[:, 256 * j:256 * j + 256]).astype(np.float16),
            "wv": np.ascontiguousarray(W_V[:, 256 * j:256 * j + 256]).astype(np.float16),
            "wo": np.ascontiguousarray(W_O[256 * j:256 * j + 256, :]).astype(np.float16),
            "resid": resid,
        })

    global _last_in_maps
    _last_in_maps = in_maps
    res = run_bass_kernel_spmd(nc, in_maps, core_ids=list(range(N_CORES)))

    out = np.empty((B, S, DM), dtype=np.float32)
    for core in range(N_CORES):
        b, j = core // 4, core % 4
        o = res.results[core]["out"]
        for c in range(NCH):
            r0 = c * 512 + j * 128
            out[b, r0:r0 + 128, :] = o[c]
    return out



# revision 21
# speedup vs baseline: 1.0908x; 1.0184x over previous
"""Trainium2 Bass kernel for nn_MultiHeadAttention_54211077210696.

8-core SPMD sharding: batch (2-way) x heads (4-way).
Core c (b = c//4, j = c%4) computes heads 4j..4j+3 of batch b.

v3 design:
  - Q^T/K^T projections in head-pair layout [128, S] (head h of pair at
    partitions h*64..h*64+64) -- no zero padding.
  - Scores via 64-contraction matmuls, row-tiled: the two heads of a pair
    run CONCURRENTLY on the PE array (tile_position derived from base
    partitions), halving score time.
  - PSUM as 3 rotating [128,1024] score slots (one kt: 2 heads x 512q)
    + 2 x [128,512] caug accumulators; exp per slot on ScalarE; scores
    stay 2 slots ahead of PV so the ACT queue never starves.
  - Softmax denominator from an all-ones block appended to V.
  - q processed in 4 chunks of 512; out-projection + fp16 ReduceScatter
    per chunk, interleaved into the next chunk's sweep; LayerNorms for
    early chunks interleaved too (rstd = exp(-0.5*ln(var+eps)) keeps Ln
    and Exp in ONE ACT table set -- no table switches ever).
  - x tensors fully resident, whole-chunk DMAs round-robined on the 3
    HWDGE queues in need-order, no ring gating.
Matmuls fp16 with fp32 PSUM accumulation.
"""

import numpy as np
from contextlib import ExitStack

import ml_dtypes

F8NP = ml_dtypes.float8_e4m3

import concourse.bass as bass  # noqa: F401  (registers bass types)
import concourse.tile as tile
from concourse import bacc, mybir
from concourse.bass_utils import run_bass_kernel_spmd

F32 = mybir.dt.float32
F16 = mybir.dt.float16
F8 = mybir.dt.float8e4
AF = mybir.ActivationFunctionType
ALU = mybir.AluOpType
PM = mybir.MatmulPerfMode
U8 = mybir.dt.uint8
ESHIFT = 3.5  # exp(s/8 - ESHIFT): softmax-invariant, keeps fp8 eo finite
LOG2E8 = 11.54156509222775  # 8*log2(e)
SCH_A = 0.125 * LOG2E8      # Schraudolph fp8e4 bits = s*SCH_A + SCH_B
SCH_B = 56.0 - ESHIFT * LOG2E8
DVE_KTS = (3, 7, 11, 14)  # kts whose exp runs on DVE (Schraudolph)

_NC = None

N_CORES = 8
GROUPS = [[0, 1, 2, 3], [4, 5, 6, 7]]
B, S, DM = 2, 2048, 1024
HL = 4          # heads per core
DT = 2          # head pairs per core
KT16 = 16       # 128-row k tiles
NCH = 4         # q chunks
QC = 512        # q chunk width
EPS = 1e-5


def _layernorm_c(nc, t, c, lnp, stp, resid_sb, eps_t):
    # ro16 rides the sync queue: the gpsimd queue blocks on in-flight
    # collectives, which would stall every DVE op queued behind this load.
    ro16 = lnp.tile([128, DM], F16, tag="ro16", name="ro16")
    nc.sync.dma_start(ro16[:], t["rs_out"][c].ap())
    of = lnp.tile([128, DM], F32, tag="of", name="of")
    nc.vector.tensor_copy(of[:], ro16[:])
    orow = lnp.tile([128, DM], F32, tag="orow", name="orow")
    nc.vector.tensor_add(orow[:], of[:], resid_sb[c][:])
    stats = stp.tile([128, 2, 6], F32, tag="st", name="st")
    for i in range(2):
        nc.vector.bn_stats(stats[:, i, :], orow[:, i * 512:(i + 1) * 512])
    mv = stp.tile([128, 2], F32, tag="mv", name="mv")
    nc.vector.bn_aggr(mv[:], stats[:])
    # rstd = exp(-0.5*ln(var+eps)): Ln and Exp share one ACT table set.
    lnv = stp.tile([128, 1], F32, tag="lnv", name="lnv")
    nc.scalar.activation(lnv[:], mv[:, 1:2], AF.Ln, bias=eps_t[:], scale=1.0)
    rstd = stp.tile([128, 1], F32, tag="rstd", name="rstd")
    nc.scalar.activation(rstd[:], lnv[:], AF.Exp, scale=-0.5)
    nc.vector.tensor_scalar(orow[:], orow[:], mv[:, 0:1], rstd[:],
                            ALU.subtract, ALU.mult)
    nc.sync.dma_start(t["out"][c][:, 0:512], orow[:, 0:512])
    nc.scalar.dma_start(t["out"][c][:, 512:1024], orow[:, 512:1024])


def _emit(nc, tc, ctx, t):
    sing = ctx.enter_context(tc.tile_pool(name="sing", bufs=1))
    eop = ctx.enter_context(tc.tile_pool(name="eop", bufs=4))
    ctp = ctx.enter_context(tc.tile_pool(name="ctp", bufs=4))
    rp = ctx.enter_context(tc.tile_pool(name="rp", bufs=2))
    ostp = ctx.enter_context(tc.tile_pool(name="ostp", bufs=2))
    lnp = ctx.enter_context(tc.tile_pool(name="lnp", bufs=2))
    stp = ctx.enter_context(tc.tile_pool(name="stp", bufs=4))
    psp = ctx.enter_context(tc.tile_pool(name="psp", bufs=1, space="PSUM"))

    # persistent SBUF tiles
    QTt = [sing.tile([128, S], F16, tag=f"qt{d}", name=f"qt{d}") for d in range(DT)]
    KTt = [sing.tile([128, S], F16, tag=f"kt{d}", name=f"kt{d}") for d in range(DT)]
    # fp8 V (+ones) for DoubleRow PV: [128 k, h*8+kp, kt-parity, 128 cols]
    # (same byte layout as the old flat (h*KT16+kt)*128 indexing)
    vaug = sing.tile([128, HL * 8, 2, 128], F8, tag="vaug")
    wo8_sb = sing.tile([128, DT, DM], F8, tag="wo8", name="wo8")
    resid_sb = [sing.tile([128, DM], F32, tag=f"res{c}", name=f"res{c}")
                for c in range(NCH)]
    eps_t = sing.tile([128, 1], F32, tag="eps")
    nbias_t = sing.tile([128, 1], F32, tag="nbias")
    wkt = [sing.tile([128, 256], F16, tag=f"wk{i}", name=f"wk{i}") for i in range(8)]
    wqt = [sing.tile([128, 256], F16, tag=f"wq{i}", name=f"wq{i}") for i in range(8)]
    wvt = [sing.tile([128, 256], F16, tag=f"wv{i}", name=f"wv{i}") for i in range(8)]
    xk_res = [sing.tile([128, S], F16, tag=f"xk{i}", name=f"xk{i}") for i in range(8)]
    xq_res = [sing.tile([128, S], F16, tag=f"xq{i}", name=f"xq{i}") for i in range(8)]
    xv_res = [sing.tile([128, S], F16, tag=f"xv{i}", name=f"xv{i}") for i in range(8)]

    nc.vector.memset(eps_t[:], EPS)
    nc.vector.memset(nbias_t[:], -ESHIFT)
    warm = sing.tile([128, 1], F32, tag="warm")
    nc.scalar.activation(warm[:], eps_t[:], AF.Exp, scale=1.0)
    nc.gpsimd.memset(vaug[:], 1.0)

    # ---- DMA prefetch: whole-chunk transfers, need-order, no gating ----
    qrot = [nc.sync, nc.scalar, nc.gpsimd]
    for i in range(8):
        nc.scalar.dma_start(wkt[i][:], t["wk"][i * 128:(i + 1) * 128, :])
    for i in range(8):
        qrot[i % 3].dma_start(xk_res[i][:], t["xkT"][i * 128:(i + 1) * 128, :])
    for i in range(8):
        nc.gpsimd.dma_start(wqt[i][:], t["wq"][i * 128:(i + 1) * 128, :])
    for i in range(8):
        qrot[i % 3].dma_start(xq_res[i][:], t["xqT"][i * 128:(i + 1) * 128, :])
    for i in range(8):
        nc.gpsimd.dma_start(wvt[i][:], t["wv"][i * 128:(i + 1) * 128, :])
    nc.gpsimd.dma_start(wo8_sb[:], t["wo8"])
    for i in range(8):
        qrot[i % 3].dma_start(xv_res[i][:], t["xvT"][i * 128:(i + 1) * 128, :])
    for c in range(NCH):
        nc.gpsimd.dma_start(resid_sb[c][:], t["resid"][c])

    # Tiny dummy collective: absorbs the ~40us CC entry barrier + stream
    # spin-up during the QKV front so the first real RS completes fast.
    nc.gpsimd.collective_compute(
        "ReduceScatter", ALU.add, replica_groups=GROUPS,
        ins=[t["brx"].ap().opt()], outs=[t["bro"].ap().opt()])

    # PSUM: 3 rotating Bp slots [128,1024] (6 banks) + 2 cg [128,512].
    def psB():
        return psp.tile([128, 1024], F32, tag="Bp", name="psB", bufs=3)

    def psC(nm="psc"):
        return psp.tile([128, 512], F32, tag="cg", name=nm, bufs=2)

    # ---- PE warmup: ~5us of junk matmuls while the first DMAs land, so
    # the HAM clock-gate opens before the K projection starts.
    junk_sb = sing.tile([128, 512], F32, tag="junk")
    nc.vector.memset(junk_sb[:], 0.0)
    junkps = psC("junkps")
    for _ in range(12):
        nc.tensor.matmul(junkps[0:1, :], eps_t[:, 0:1], junk_sb[:])

    # ---- K / Q projections: 8 chains over all 8 banks ----
    def qk_sweep(xres, wt, dst):
        pb = [psB() for _ in range(3)]
        pc = [psC() for _ in range(2)]

        def chain(j):
            if j < 6:
                return pb[j // 2][:, (j % 2) * 512:(j % 2) * 512 + 512]
            return pc[j - 6][:, :]

        for dmc in range(8):
            xc = xres[dmc]
            st = dict(start=(dmc == 0), stop=(dmc == 7))
            for d in range(DT):
                for sc in range(4):
                    nc.tensor.matmul(chain(d * 4 + sc), wt[dmc][:, d * 128:(d + 1) * 128],
                                     xc[:, sc * 512:(sc + 1) * 512], **st)
        for d in range(DT):
            for sc in range(4):
                nc.vector.tensor_copy(dst[d][:, sc * 512:(sc + 1) * 512],
                                      chain(d * 4 + sc))

    qk_sweep(xk_res, wkt, KTt)
    qk_sweep(xq_res, wqt, QTt)

    # ---- V projection: 2 waves x 8 s-tile chains ----
    for wave in range(2):
        pb = [psB() for _ in range(3)]
        pc = [psC("pscv") for _ in range(2)]

        def vchain(i):
            if i < 6:
                return pb[i // 2][:, (i % 2) * 512:(i % 2) * 512 + 256]
            return pc[i - 6][:, 0:256]

        for dmc in range(8):
            for i in range(8):
                st = wave * 8 + i
                nc.tensor.matmul(vchain(i), xv_res[dmc][:, st * 128:(st + 1) * 128],
                                 wvt[dmc][:], start=(dmc == 0), stop=(dmc == 7))
        for i in range(8):
            st = wave * 8 + i
            src = vchain(i)
            for h in range(HL):
                nc.vector.tensor_copy(
                    vaug[:, h * 8 + st // 2, st % 2, 64:128],
                    src[:, h * 64:h * 64 + 64])

    # ---- attention: 4 chunks x 2 pairs; per kt one [128,1024] slot.
    # eo is fp8 in kt-pair parity layout [128, 2, 1024]; PV is a DoubleRow
    # matmul per (kt-pair, head): contraction 256 at 2x fp8 rate.
    def emit_pv(p, caug, eo, kp):
        for h in range(2):
            nc.tensor.matmul(caug[h][:, :],
                             vaug[:, (p * 2 + h) * 8 + kp, :, :],
                             eo[:, :, h * 512:(h + 1) * 512],
                             start=(kp == 0), stop=(kp == KT16 // 2 - 1),
                             perf_mode=PM.DoubleRow)

    def sweep(c, p, ctb, hooks, finish_prev):
        """Emit one (chunk, pair) kt sweep. exp split ACT/DVE: DVE kts use a
        Schraudolph bit-trick straight to fp8e4 bits (u8 round-to-nearest).
        Writes ctb[:, p, :] when the returned finish() has been emitted.
        """
        qb = c * QC
        caug = [psC(f"caug{h}") for h in range(2)]
        pend = []
        eo = None
        hooks = dict(hooks)
        for kt in range(KT16):
            ps = psB()
            for h in range(2):
                nc.tensor.matmul(
                    ps[:, h * 512:(h + 1) * 512],
                    KTt[p][h * 64:(h + 1) * 64, kt * 128:(kt + 1) * 128],
                    QTt[p][h * 64:(h + 1) * 64, qb:qb + QC])
            if kt % 2 == 0:
                eo = eop.tile([128, 2, 1024], F8, tag="eo", name="eo")
            if kt in DVE_KTS:
                nc.vector.tensor_scalar(eo[:, kt % 2, :].bitcast(U8), ps[:],
                                        SCH_A, SCH_B, ALU.mult, ALU.add)
            else:
                nc.scalar.activation(eo[:, kt % 2, :], ps[:], AF.Exp,
                                     bias=nbias_t[:], scale=0.125)
            if kt % 2 == 1:
                pend.append((eo, kt // 2))
            if kt == 1 and finish_prev is not None:
                # two slots of lookahead are already queued on ACT, so the
                # previous sweep's PE flush hides under them
                finish_prev()
            if len(pend) > 1:
                emit_pv(p, caug, *pend.pop(0))
            if kt in hooks:
                for fn in hooks[kt]:
                    fn()

        def finish():
            for pe in pend:
                emit_pv(p, caug, *pe)
            # normalize: rows 0..63 of caug hold the softmax denominator
            # (ones in vaug cols 0:64 -- recip_approx_fast needs base-0 input)
            for h in range(2):
                rt2 = rp.tile([64, QC], F32, tag="rt2", name="rt2")
                nc.vector.reciprocal_approx_fast(rt2[:], caug[h][0:64, :])
                nc.vector.tensor_mul(ctb[h * 64:(h + 1) * 64, :, p, :],
                                     caug[h][64:128, :], rt2[:])
        return finish

    def outproj_piece(c, ctb, qt, on_act):
        # one fp8 DoubleRow matmul contracts both pairs (256 rows) at once
        po = psB()
        for dmc in range(2):
            nc.tensor.matmul(po[:, dmc * 512:(dmc + 1) * 512], ctb[:, qt, :, :],
                             wo8_sb[:, :, dmc * 512:(dmc + 1) * 512],
                             perf_mode=PM.DoubleRow)
        ost = ostp.tile([128, 1024], F8, tag="ost", name="ost")
        if on_act:
            nc.scalar.activation(ost[:], po[:], AF.Copy, bias=0.0, scale=1.0)
        else:
            nc.vector.tensor_copy(ost[:], po[:])
        nc.sync.dma_start(t["rs_in"][c][qt * 128:(qt + 1) * 128, :], ost[:])

    def fire_rs(c):
        nc.gpsimd.collective_compute(
            "ReduceScatter", ALU.add, replica_groups=GROUPS,
            ins=[t["rs_in"][c].ap().opt()], outs=[t["rs_out"][c].ap().opt()])

    ctb_prev = None
    c_prev = None
    fin_prev = None
    for c in range(NCH):
        # [k?, qt, pair, 128]: qt slice contiguous for the DR lhsT
        ctb_cur = ctp.tile([128, 4, DT, 128], F8, tag="ctb", name="ctb")
        for p in range(DT):
            hooks = []
            if p == 0 and ctb_prev is not None:
                cp, cc = c_prev, ctb_prev
                hooks = [(2, [lambda cp=cp, cc=cc: outproj_piece(cp, cc, 0, True)]),
                         (5, [lambda cp=cp, cc=cc: outproj_piece(cp, cc, 1, False)]),
                         (8, [lambda cp=cp, cc=cc: outproj_piece(cp, cc, 2, True)]),
                         (11, [lambda cp=cp, cc=cc: outproj_piece(cp, cc, 3, False)]),
                         (13, [lambda cp=cp: fire_rs(cp)])]
            fin_prev = sweep(c, p, ctb_cur, hooks, fin_prev)
        if c > 0:
            nc.scalar.dma_start(t["dbg_ctb"][c - 1], ctb_prev[:])
        ctb_prev, c_prev = ctb_cur, c
    fin_prev()
    nc.scalar.dma_start(t["dbg_ctb"][NCH - 1], ctb_prev[:])

    # ---- tail: last chunk out-proj, fire RS_3, then all LayerNorms,
    # batched: prefetch every rs_out shard first, residual-add in place
    # into resid_sb, then ALL Ln's followed by ALL Exp's so the ACT table
    # switches twice total instead of per-LN. wait_until pins this past
    # the loop so the scheduler cannot hoist it.
    for qt in range(4):
        outproj_piece(c_prev, ctb_prev, qt, qt % 2 == 0)
    fire_rs(c_prev)
    with tc.tile_wait_until(2.0):
        ro16s, mvs = [], []
        for c in range(NCH):
            ro = lnp.tile([128, DM], F8, tag="ro8", name="ro8", bufs=4)
            nc.sync.dma_start(ro[:], t["rs_out"][c].ap())
            ro16s.append(ro)
        for c in range(NCH):
            nc.vector.tensor_add(resid_sb[c][:], ro16s[c][:], resid_sb[c][:])
            stats = stp.tile([128, 2, 6], F32, tag="st", name="st")
            for i in range(2):
                nc.vector.bn_stats(stats[:, i, :],
                                   resid_sb[c][:, i * 512:(i + 1) * 512])
            mv = stp.tile([128, 2], F32, tag="mv", name="mv")
            nc.vector.bn_aggr(mv[:], stats[:])
            mvs.append(mv)
        # chunks 0-2: three Ln's into one tile + ONE 3-wide Exp (2 table
        # switches, not 6), finished while RS_3 is still in flight; then
        # chunk 3 alone once its shard lands.
        lnv3 = stp.tile([128, 4], F32, tag="lnv", name="lnv3")
        for c in range(3):
            nc.scalar.activation(lnv3[:, c:c + 1], mvs[c][:, 1:2], AF.Ln,
                                 bias=eps_t[:], scale=1.0)
        rstd3 = stp.tile([128, 4], F32, tag="rstd", name="rstd3")
        nc.scalar.activation(rstd3[:, 0:3], lnv3[:, 0:3], AF.Exp, scale=-0.5)
        for c in range(3):
            nc.vector.tensor_scalar(resid_sb[c][:], resid_sb[c][:],
                                    mvs[c][:, 0:1], rstd3[:, c:c + 1],
                                    ALU.subtract, ALU.mult)
            nc.sync.dma_start(t["out"][c][:, 0:512], resid_sb[c][:, 0:512])
            nc.scalar.dma_start(t["out"][c][:, 512:1024],
                                resid_sb[c][:, 512:1024])
        lnv = stp.tile([128, 1], F32, tag="lnv", name="lnv")
        nc.scalar.activation(lnv[:], mvs[3][:, 1:2], AF.Ln,
                             bias=eps_t[:], scale=1.0)
        rstd = stp.tile([128, 1], F32, tag="rstd", name="rstd")
        nc.scalar.activation(rstd[:], lnv[:], AF.Exp, scale=-0.5)
        nc.vector.tensor_scalar(resid_sb[3][:], resid_sb[3][:],
                                mvs[3][:, 0:1], rstd[:],
                                ALU.subtract, ALU.mult)
        nc.sync.dma_start(t["out"][3][:, 0:512], resid_sb[3][:, 0:512])
        nc.scalar.dma_start(t["out"][3][:, 512:1024], resid_sb[3][:, 512:1024])


def _build():
    nc = bacc.Bacc("TRN2", target_bir_lowering=False, debug=False,
                   num_devices=N_CORES)
    t = {}
    for name in ("xqT", "xkT", "xvT"):
        t[name] = nc.dram_tensor(name, [DM, S], F16, kind="ExternalInput").ap()
    for name in ("wq", "wk", "wv"):
        t[name] = nc.dram_tensor(name, [DM, 256], F16, kind="ExternalInput").ap()
    t["wo8"] = nc.dram_tensor("wo8", [128, DT, DM], F8, kind="ExternalInput").ap()
    t["resid"] = nc.dram_tensor("resid", [NCH, 128, DM], F32, kind="ExternalInput").ap()
    t["out"] = nc.dram_tensor("out", [NCH, 128, DM], F32, kind="ExternalOutput").ap()
    t["rs_in"] = [nc.dram_tensor(f"rs_in{c}", [512, DM], F8) for c in range(NCH)]
    t["rs_out"] = [nc.dram_tensor(f"rs_out{c}", [128, DM], F8) for c in range(NCH)]
    t["dbg_ctb"] = nc.dram_tensor("dbg_ctb", [NCH, 128, 4, DT, 128], F8,
                                  kind="ExternalOutput").ap()
    t["brx"] = nc.dram_tensor("brx", [8, 128], F16)
    t["bro"] = nc.dram_tensor("bro", [2, 128], F16)

    with tile.TileContext(nc) as tc:
        with ExitStack() as ctx:
            _emit(nc, tc, ctx, t)
    nc.compile()
    return nc


def kernel(input_Q, input_K, input_V, W_Q, W_K, W_V, W_O):
    global _NC
    if _NC is None:
        _NC = _build()
    nc = _NC

    input_Q = np.asarray(input_Q, dtype=np.float32)
    input_K = np.asarray(input_K, dtype=np.float32)
    input_V = np.asarray(input_V, dtype=np.float32)
    W_Q = np.asarray(W_Q, dtype=np.float32)
    W_K = np.asarray(W_K, dtype=np.float32)
    W_V = np.asarray(W_V, dtype=np.float32)
    W_O = np.asarray(W_O, dtype=np.float32)

    xT = {}
    for nm, x in (("q", input_Q), ("k", input_K), ("v", input_V)):
        for b in range(B):
            xT[nm, b] = np.ascontiguousarray(x[b].T).astype(np.float16)
    in_maps = []
    for core in range(N_CORES):
        b, j = core // 4, core % 4
        resid = np.empty((NCH, 128, DM), dtype=np.float32)
        for c in range(NCH):
            r0 = c * 512 + j * 128
            resid[c] = input_Q[b, r0:r0 + 128, :]
        in_maps.append({
            "xqT": xT["q", b], "xkT": xT["k", b], "xvT": xT["v", b],
            "wq": np.ascontiguousarray(W_Q[:, 256 * j:256 * j + 256]).astype(np.float16),
            "wk": np.ascontiguousarray(W_K[:, 256 * j:256 * j + 256]).astype(np.float16),
            "wv": np.ascontiguousarray(W_V[:, 256 * j:256 * j + 256]).astype(np.float16),
            "wo8": np.ascontiguousarray(
                W_O[256 * j:256 * j + 256, :].reshape(DT, 128, DM)
                .transpose(1, 0, 2)).astype(F8NP),
            "resid": resid,
        })

    global _last_in_maps
    _last_in_maps = in_maps
    res = run_bass_kernel_spmd(nc, in_maps, core_ids=list(range(N_CORES)))

    out = np.empty((B, S, DM), dtype=np.float32)
    for core in range(N_CORES):
        b, j = core // 4, core % 4
        o = res.results[core]["out"]
        for c in range(NCH):
            r0 = c * 512 + j * 128
            out[b, r0:r0 + 128, :] = o[c]
    return out

